# revision 22
# baseline (speedup 1.0000x reference)
"""Trainium2 Bass kernel for the masked-logsumexp multi-branch loss (final).

Problem: logit [524288, 128] f32, target [524288, 128] int32 (multi-hot 0/1).
Per row r (is_null = target[r,0]):
  branch1 (is_null): lse(all) - x0
  branch2: (n_pos*lse({0} u pos) - sum_pos_logit)/max(n_pos,1) + lse(neg u {0}) - x0
Output: scalar mean over all rows. Data-parallel over 8 NeuronCores (65536
rows/core), rows on SBUF partitions; per-row sums shared across the three
masked logsumexps: S_E, S_ME, S_MX, S_M plus class-0 extracts.

Host re-encoding (casts/slices/affine of single tensors only): logit/target
-> bf16; class-0 columns as separate [B] tensors; mask column 0 zeroed
(ships as (mask-sans-col0, col0), mirroring the reference's pos_no0); and
m_enc = (target-1)*32 in bf16 ({0,-32}, col0=-32). Row mapping is
partition-major (row = p*512 + s*NB + n) so column tensors and the lo
output are contiguous per partition.

Per [128, 16*128] super-tile (NB=16, 32/core):
  DMA(HWDGE): xb -> W5[:,4]; 2nd read of xb's upper half into Xp.
  DMA(SWDGE): mask int8 -> bf16 casting load into W5[:,0] (saves 8MB/core
              of HBM); Xp += m_enc (compute-copy add, 4KB/partition
              descriptors) -- an independent chain joining only at the fold.
  ScalarE  : exp(xb) -> W5[:,3]; exp(Xp) -> W5[:,1] upper half
             (exp(x+m_enc) == M*exp(x) exactly on kept terms, ~1e-14 leak
             on masked terms).
  VectorE  : lower half: one 2-wide bf16 2x TT for [M*E|M*x]; upper half:
             M*x only (the M*E product is what the SWDGE chain replaced);
             then a shared 2x fold chain over [mb|M*E|M*x|E] to width 2,
             landing in the bf16 stat array. The CCE covers only half the
             stream because its RMW runs ~4x below line rate: full-stream
             CCE starves the pipeline (measured 263-287us), zero-CCE leaves
             DVE product work on the floor (250us); half splits the load so
             per-ST DMA time stays under per-ST DVE time.
  (GPSIMD compute and TensorE unusable: Pool shares the DVE SBUF port and
   is ~2x slower per element; PE only contracts the partition axis and PSUM
   accumulation is matmul-only; tensor_reduce/pool/TTR/custom-DVE run at 1x
   < the 2x fold chain.)
Super-tile 0 is emitted in progressive pieces (2/2/4/8 blocks) and STs 1-2
stay on the classic product path so the SWDGE chains engage with full
lookahead. Combine runs per chunk in bf16, split E0-early / V1+A1 at the
trigger / V2 one ST later so the DVE never waits on the ACT logs; each
chunk streams its bf16 losses out immediately. Final mean on host (f64).

Measured on trn2 (8 cores): ~238us HW exec (vs 262.5us baseline; Vector
~211us busy, ~10.6us engine preamble, ~3us gaps, ~5us tail), output rel
err 3.3e-4 vs the 2e-2 tolerance (validated bit-exact in numpy first).
Also rejected on measurement: NB=32 tiles (DVE throughput -4% at 8KB FD),
SWDGE int8 mask-fold (descriptor-bound at 8-64B chunks: 633us), CCE on
5/8 of the stream with or without the int8 mask (241-246us: the SWDGE
cost grows faster than the DVE saving past the half split).

Note: this container's walrus accepts one sync-wait per instruction, so
_split_sync_waits() rewrites the Tile-scheduled BIR accordingly.
"""
import numpy as np
import ml_dtypes

import concourse.bass as bass
import concourse.tile as tile
from concourse import mybir
from concourse.bass_utils import run_bass_kernel_spmd

B = 524288
C = 128
NCORES = 8
RPC = B // NCORES  # rows per core = 65536
P = 128  # partitions
NB = 16  # class-blocks per super-tile -> [128, NB*128] tiles
ST = RPC // (P * NB)  # super-tiles per core = 16
NSTATS = ST * NB  # stat columns per core = 512

FP32 = mybir.dt.float32
BF16 = mybir.dt.bfloat16
I8 = mybir.dt.int8
ALU = mybir.AluOpType
AF = mybir.ActivationFunctionType


def _build_kernel(tc: tile.TileContext, lo, xb, mb, x0c, m0c, menc):
    nc = tc.nc
    # row = p*(ST*NB) + s*NB + n ; per (s, p): NB*C contiguous bf16 = 8KB lines
    Xd = xb.rearrange("(p s n) c -> s p (n c)", s=ST, n=NB)
    Md = mb.rearrange("(p s n) c -> s p (n c)", s=ST, n=NB)
    LOd = lo.rearrange("(p s n) -> p s n", s=ST, n=NB)
    X0d = x0c.rearrange("(p w) -> p w", w=NSTATS)
    Ed = menc.rearrange("(p s n) c -> s p (n c)", s=ST, n=NB)
    M0d = m0c.rearrange("(p w) -> p w", w=NSTATS)

    import contextlib

    with contextlib.ExitStack() as ctx:
        stats = ctx.enter_context(tc.tile_pool(name="stats", bufs=1))
        work = ctx.enter_context(tc.tile_pool(name="work", bufs=3))
        fpool = ctx.enter_context(tc.tile_pool(name="fpool", bufs=2))
        small = ctx.enter_context(tc.tile_pool(name="small", bufs=2))
        xpool = ctx.enter_context(tc.tile_pool(name="xpool", bufs=4))

        # Persistent per-core stat arrays: S_ALL[q] for q in [M, ME, MX, E]
        S_ALL = stats.tile([P, 4, ST, NB, 2], BF16)  # fold stops at width 2
        X0 = stats.tile([P, NSTATS], BF16)
        M0 = stats.tile([P, NSTATS], BF16)

        lot = small.tile([P, NSTATS], BF16, tag="lot")
        warm = stats.tile([P, 1], BF16)
        # dependency-free first ACT op: forces the exp/ln table load during
        # the DMA ramp instead of stalling the first real exp
        nc.scalar.activation(out=warm, in_=warm, func=AF.Exp)


        # ---- combine (emitted interleaved with the ST loop) ----
        CH_END = [8, 16, 24, 30, 32]  # trigger after this many STs
        WMAX = 8 * NB

        def flat(t):
            return t.rearrange("p a b -> p (a b)")

        def chunk_tiles(ch):
            lo_c = (CH_END[ch - 1] if ch else 0) * NB
            hi_c = CH_END[ch] * NB
            W = hi_c - lo_c
            sl = slice(lo_c, hi_c)

            def tl(tag):
                t = small.tile([P, WMAX], BF16, tag=tag, name=f"cmb{ch}-{tag}")
                return t[:, :W]

            SQ = S_ALL.rearrange("p q s n two -> p q (s n) two")[:, :, sl]
            SQs = small.tile([P, 4, WMAX], BF16, tag="sqs", name=f"cmb{ch}-sqs")[
                :, :, :W
            ]
            return {
                "sl": sl,
                "SQ": SQ,
                "SQs": SQs,
                "sM": SQs[:, 0],
                "sME": SQs[:, 1],
                "sMX": SQs[:, 2],
                "sE": SQs[:, 3],
                "x0": X0[:, sl],
                "m0": M0[:, sl],
                "E0": tl("c0"),
                "t_a": tl("c1"),
                "s_pos": tl("c2"),
                "lse_all": tl("c3"),
                "lse_pos": tl("c4"),
                "lse_neg": tl("c5"),
                "rinv": tl("c7"),
                "s_neg": tl("c8"),
                "npc": tl("c9"),
                "t_b": tl("c11"),
            }

        def combine_pre(t):
            # one ST before the trigger: E0 only (needs just the prefetch)
            nc.scalar.activation(out=t["E0"], in_=t["x0"], func=AF.Exp)

        def combine_v1a1(t):
            # ---- V1: finish the fold (width 2 -> 1) for all four stats
            nc.vector.tensor_add(t["SQs"], t["SQ"][:, :, :, 0], t["SQ"][:, :, :, 1])
            # mask col0 is zeroed on host, so sums exclude class 0
            nc.vector.tensor_add(t["s_pos"], t["sME"], t["E0"])
            nc.vector.tensor_sub(t["s_neg"], t["sE"], t["sME"])
            nc.vector.tensor_tensor(
                out=t["s_neg"], in0=t["s_neg"], in1=t["E0"], op=ALU.max
            )
            nc.vector.tensor_scalar_max(t["npc"], t["sM"], 1.0)
            # ---- A1: the logs, batched
            nc.scalar.activation(out=t["lse_all"], in_=t["sE"], func=AF.Ln)
            nc.scalar.activation(out=t["lse_pos"], in_=t["s_pos"], func=AF.Ln)
            nc.scalar.activation(out=t["lse_neg"], in_=t["s_neg"], func=AF.Ln)
            nc.scalar.activation(out=t["rinv"], in_=t["npc"], func=AF.Ln)
            nc.scalar.activation(out=t["rinv"], in_=t["rinv"], func=AF.Exp, scale=-1.0)

        def combine_v2(ch, t):
            # ---- V2: finish (ACT logs are ~one ST old by now -> no stall)
            ta, tb = t["t_a"], t["t_b"]
            nc.vector.tensor_mul(ta, t["sM"], t["lse_pos"])
            nc.vector.tensor_sub(ta, ta, t["sMX"])
            nc.vector.tensor_mul(ta, ta, t["rinv"])
            nc.vector.tensor_add(ta, ta, t["lse_neg"])  # acc (loss_full sans -x0)
            # lo = acc + m0*(lse_all - acc) - x0   (m0 in {0,1})
            nc.vector.tensor_sub(tb, t["lse_all"], ta)
            nc.vector.tensor_mul(tb, t["m0"], tb)
            nc.vector.tensor_add(ta, ta, tb)
            nc.vector.tensor_sub(lot[:, t["sl"]], ta, t["x0"])
            # stream this chunk's losses out now (contiguous per partition)
            s0 = CH_END[ch - 1] if ch else 0
            nc.sync.dma_start(
                out=LOd[:, s0 : CH_END[ch]],
                in_=lot.rearrange("p (s n) -> p s n", n=NB)[:, s0 : CH_END[ch]],
            )

        def emit_compute(s, W5f, Xp, n0, nn):
            """Compute on row-blocks [n0, n0+nn) of super-tile s."""
            W5 = W5f[:, :, n0 : n0 + nn]
            et = W5[:, 3]
            xbf = W5[:, 4]

            # ScalarE: exp (only needs the xb DMA)
            nc.scalar.activation(out=et, in_=xbf, func=AF.Exp)

            if Xp is None:
                # classic path: both products in one bf16 2x TT
                nc.vector.tensor_mul(
                    W5[:, 1:3], W5[:, 3:5], W5[:, 0:1].broadcast_to([P, 2, nn, C])
                )
            else:
                # hybrid: first half classic 2-wide product; second half gets
                # M*E as exp(x+m_enc) with x' built by a SWDGE compute-add
                # (keeps the extra DMA under the per-ST DVE period)
                h = nn // 2
                nc.vector.tensor_mul(
                    W5[:, 1:3, n0 : n0 + h],
                    W5[:, 3:5, n0 : n0 + h],
                    W5[:, 0:1, n0 : n0 + h].broadcast_to([P, 2, h, C]),
                )
                nc.vector.tensor_mul(
                    W5[:, 2, n0 + h : n0 + nn],
                    W5[:, 0, n0 + h : n0 + nn],
                    W5[:, 4, n0 + h : n0 + nn],
                )
                nc.scalar.activation(
                    out=W5[:, 1, n0 + h : n0 + nn], in_=Xp, func=AF.Exp
                )

            # single fold chain over all four quantities (bf16 2x adds)
            Q = W5[:, 0:4]
            f1 = fpool.tile([P, 4, NB, C // 2], BF16, tag="f1", name="f1")[:, :, n0 : n0 + nn]
            f2 = fpool.tile([P, 4, NB, C // 4], BF16, tag="f2", name="f2")[:, :, n0 : n0 + nn]
            f3 = fpool.tile([P, 4, NB, C // 8], BF16, tag="f3", name="f3")[:, :, n0 : n0 + nn]
            f4 = fpool.tile([P, 4, NB, C // 16], BF16, tag="f4", name="f4")[:, :, n0 : n0 + nn]
            f5 = fpool.tile([P, 4, NB, C // 32], BF16, tag="f5", name="f5")[:, :, n0 : n0 + nn]
            nc.vector.tensor_add(f1, Q[:, :, :, 0 : C // 2], Q[:, :, :, C // 2 : C])
            nc.vector.tensor_add(
                f2, f1[:, :, :, 0 : C // 4], f1[:, :, :, C // 4 : C // 2]
            )
            nc.vector.tensor_add(
                f3, f2[:, :, :, 0 : C // 8], f2[:, :, :, C // 8 : C // 4]
            )
            nc.vector.tensor_add(
                f4, f3[:, :, :, 0 : C // 16], f3[:, :, :, C // 16 : C // 8]
            )
            nc.vector.tensor_add(
                f5, f4[:, :, :, 0 : C // 32], f4[:, :, :, C // 32 : C // 16]
            )
            nc.vector.tensor_add(
                S_ALL[:, :, s, n0 : n0 + nn],
                f5[:, :, :, 0 : C // 64],
                f5[:, :, :, C // 64 : C // 32],
            )

        pending = None  # (ch, tiles) awaiting V2 emission
        tiles_by_ch = {}
        for s in range(ST):
            W5f = work.tile([P, 5, NB, C], BF16, tag="W5", name="W5")
            Xs = Xd[s].rearrange("p (n c) -> p n c", c=C)
            Ms = Md[s].rearrange("p (n c) -> p n c", c=C)
            Es = Ed[s].rearrange("p (n c) -> p n c", c=C)
            if s == 0:
                # progressive first super-tile: tiny DMAs so ACT/DVE start
                # as soon as the first 128KB lands; classic product path
                n0 = 0
                for nn in (2, 2, 4, 8):
                    nc.sync.dma_start(
                        out=W5f[:, 4, n0 : n0 + nn], in_=Xs[:, n0 : n0 + nn]
                    )
                    nc.gpsimd.dma_start(
                        out=W5f[:, 0, n0 : n0 + nn], in_=Ms[:, n0 : n0 + nn]
                    )
                    emit_compute(0, W5f, None, n0, nn)
                    n0 += nn
                # class-0 column prefetch (1KB/partition, contiguous), needed
                # first by chunk 0's E0 at s = CH_END[0]-2
                nc.sync.dma_start(out=X0, in_=X0d)
                nc.sync.dma_start(out=M0, in_=M0d)
            elif s <= 2:
                # keep the classic path while the pipeline warms; the SWDGE
                # x' chains engage from s=3 with full lookahead
                nc.sync.dma_start(out=W5f[:, 4], in_=Xs)
                nc.gpsimd.dma_start(out=W5f[:, 0], in_=Ms)
                emit_compute(s, W5f, None, 0, NB)
            else:
                H = NB // 2
                Xp = xpool.tile([P, H, C], BF16, tag="xp", name="xp")
                nc.sync.dma_start(out=W5f[:, 4], in_=Xs)
                nc.gpsimd.dma_start(out=W5f[:, 0], in_=Ms)
                nc.sync.dma_start(out=Xp, in_=Xs[:, H:NB])
                nc.gpsimd.dma_start(out=Xp, in_=Es[:, H:NB], accum_op=ALU.add)
                emit_compute(s, W5f, Xp, 0, NB)
            if pending is not None:
                combine_v2(*pending)
                pending = None
            if (s + 2) in CH_END:
                ch = CH_END.index(s + 2)
                tiles_by_ch[ch] = chunk_tiles(ch)
                combine_pre(tiles_by_ch[ch])
            if (s + 1) in CH_END:
                ch = CH_END.index(s + 1)
                if ch not in tiles_by_ch:  # back-to-back chunks
                    tiles_by_ch[ch] = chunk_tiles(ch)
                    combine_pre(tiles_by_ch[ch])
                t = tiles_by_ch.pop(ch)
                combine_v1a1(t)
                if s + 1 == ST:
                    combine_v2(ch, t)
                else:
                    pending = (ch, t)
        assert pending is None


def _split_sync_waits(nc):
    """The container's walrus accepts at most ONE sync-wait command per
    instruction (the TPB EVENTS struct has a single wait slot). Tile emits
    instructions with N waits; rewrite each so the extra waits ride on
    same-engine NoOps inserted immediately before (engine program order makes
    this semantically identical)."""
    for f in nc.m.functions:
        for blk in f.blocks:
            insts = blk.instructions
            out = []
            changed = False
            for inst in insts:
                si = inst.sync_info
                waits = list(si.on_wait) if (si is not None and si.on_wait) else []
                if len(waits) > 1:
                    changed = True
                    for k, w in enumerate(waits[:-1]):
                        nop = mybir.InstNoOp(name=f"{inst.name}-w{k}", ins=[], outs=[])
                        nop.engine = inst.engine
                        nop.sync_info = mybir.SyncInfo(on_wait=[w], on_update=[])
                        out.append(nop)
                    inst.sync_info = mybir.SyncInfo(
                        on_wait=[waits[-1]],
                        on_update=list(si.on_update) if si.on_update else [],
                    )
                out.append(inst)
            if changed:
                blk.instructions = out


_NC_CACHE = None
SPLIT_WAITS = True


def _get_nc():
    global _NC_CACHE
    if _NC_CACHE is None:
        nc = bass.Bass()
        xb = nc.declare_dram_parameter("xb", [RPC, C], BF16, isOutput=False)
        mb = nc.declare_dram_parameter("mb", [RPC, C], I8, isOutput=False)
        x0c = nc.declare_dram_parameter("x0c", [RPC], BF16, isOutput=False)
        m0c = nc.declare_dram_parameter("m0c", [RPC], BF16, isOutput=False)
        menc = nc.declare_dram_parameter("menc", [RPC, C], I8, isOutput=False)
        lo = nc.declare_dram_parameter("lo", [RPC], BF16, isOutput=True)
        with tile.TileContext(nc) as tc:
            _build_kernel(tc, lo, xb, mb, x0c, m0c, menc)
        if SPLIT_WAITS:
            _split_sync_waits(nc)
        _NC_CACHE = nc
    return _NC_CACHE


def _prep_inputs(logit, target):
    """Host-side re-encoding (dtype casts + column slice/zero only)."""
    xb = logit.astype(ml_dtypes.bfloat16)
    mb = target.astype(np.int8)
    x0c = np.ascontiguousarray(xb[:, 0])
    m0c = target[:, 0].astype(ml_dtypes.bfloat16)
    mb[:, 0] = 0
    menc = ((target - 1) * 32).astype(np.int8)
    menc[:, 0] = -32
    return xb, mb, x0c, m0c, menc


def _in_maps(xb, mb, x0c, m0c, menc):
    return [
        {
            "xb": xb[i * RPC : (i + 1) * RPC],
            "mb": mb[i * RPC : (i + 1) * RPC],
            "x0c": x0c[i * RPC : (i + 1) * RPC],
            "m0c": m0c[i * RPC : (i + 1) * RPC],
            "menc": menc[i * RPC : (i + 1) * RPC],
        }
        for i in range(NCORES)
    ]


def kernel(**inputs) -> np.ndarray:
    logit = np.ascontiguousarray(np.asarray(inputs["logit"], dtype=np.float32))
    target = np.ascontiguousarray(np.asarray(inputs["target"], dtype=np.int32))
    assert logit.shape == (B, C) and target.shape == (B, C)

    nc = _get_nc()
    res = run_bass_kernel_spmd(
        nc, _in_maps(*_prep_inputs(logit, target)), core_ids=list(range(NCORES))
    )
    lo = np.concatenate(
        [np.asarray(r["lo"]).reshape(-1) for r in res.results]
    ).astype(np.float32)
    return np.array(np.mean(lo, dtype=np.float64), dtype=np.float32)


# revision 23
# speedup vs baseline: 1.0221x; 1.0221x over previous
"""Trainium2 Bass kernel for the masked-logsumexp multi-branch loss (final).

Problem: logit [524288, 128] f32, target [524288, 128] int32 (multi-hot 0/1).
Per row r (is_null = target[r,0]):
  branch1 (is_null): lse(all) - x0
  branch2: (n_pos*lse({0} u pos) - sum_pos_logit)/max(n_pos,1) + lse(neg u {0}) - x0
Output: scalar mean over all rows. Data-parallel over 8 NeuronCores (65536
rows/core), rows on SBUF partitions; per-row sums shared across the three
masked logsumexps: S_E, S_ME, S_MX, S_M plus class-0 extracts.

Host re-encoding (casts/slices/affine of single tensors only): logit/target
-> bf16; class-0 columns as separate [B] tensors; mask column 0 zeroed
(ships as (mask-sans-col0, col0), mirroring the reference's pos_no0); and
m_enc = (target-1)*32 in int8 ({0,-32}, col0=-32). Row mapping is
partition-major (row = p*512 + s*NB + n) so column tensors and the lo
output are contiguous per partition.

Per [128, 16*128] super-tile (NB=16, 32/core):
  DMA(HWDGE): xb -> W5[:,4]; 2nd read of xb's upper half into Xp.
  DMA(SWDGE): mask int8 -> bf16 casting load into W5[:,0] (saves 8MB/core
              of HBM); Xp += m_enc (compute-copy cast+add from int8,
              4KB/partition descriptors) -- an independent chain joining
              only at the fold.
  ScalarE  : exp(xb) -> W5[:,3]; exp(Xp) -> W5[:,1] upper half
             (exp(x+m_enc) == M*exp(x) exactly on kept terms, ~1e-14 leak
             on masked terms).
  VectorE  : lower half: one 2-wide bf16 2x TT for [M*E|M*x]; upper half:
             M*x only (the M*E product is what the SWDGE chain replaced);
             then a shared 2x fold chain over [mb|M*E|M*x|E] to width 2,
             landing in the bf16 stat array. The CCE covers only half the
             stream because its RMW runs ~4x below line rate: full-stream
             CCE starves the pipeline (measured 263-287us), zero-CCE leaves
             DVE product work on the floor (250us); half splits the load so
             per-ST DMA time stays under per-ST DVE time.
  (GPSIMD compute and TensorE unusable: Pool shares the DVE SBUF port and
   is ~2x slower per element; PE only contracts the partition axis and PSUM
   accumulation is matmul-only; tensor_reduce/pool/TTR/custom-DVE run at 1x
   < the 2x fold chain.)
Super-tile 0 is emitted in progressive pieces (2/2/4/8 blocks) and STs 1-2
stay on the classic product path so the SWDGE chains engage with full
lookahead. Combine runs per chunk in bf16, split E0-early / V1+A1 at the
trigger / V2 one ST later so the DVE never waits on the ACT logs; each
chunk streams its bf16 losses out immediately. Final mean on host (f64).

Measured on trn2 (8 cores): ~238-242us HW exec (best 237.9us) (vs 262.5us baseline; Vector
~211us busy, ~10.6us engine preamble, ~3us gaps, ~5us tail), output rel
err 3.3e-4 vs the 2e-2 tolerance (validated bit-exact in numpy first).
Also rejected on measurement: NB=32 tiles (DVE throughput -4% at 8KB FD),
SWDGE int8 mask-fold (descriptor-bound at 8-64B chunks: 633us), CCE on
5/8 of the stream with or without the int8 mask (241-246us: the SWDGE
cost grows faster than the DVE saving past the half split), and a 4th
Xp buffer (244us).

Note: this container's walrus accepts one sync-wait per instruction, so
_split_sync_waits() rewrites the Tile-scheduled BIR accordingly.
"""
import numpy as np
import ml_dtypes

import concourse.bass as bass
import concourse.tile as tile
from concourse import mybir
from concourse.bass_utils import run_bass_kernel_spmd

B = 524288
C = 128
NCORES = 8
RPC = B // NCORES  # rows per core = 65536
P = 128  # partitions
NB = 16  # class-blocks per super-tile -> [128, NB*128] tiles
ST = RPC // (P * NB)  # super-tiles per core = 16
NSTATS = ST * NB  # stat columns per core = 512

FP32 = mybir.dt.float32
BF16 = mybir.dt.bfloat16
I8 = mybir.dt.int8
ALU = mybir.AluOpType
AF = mybir.ActivationFunctionType


def _build_kernel(tc: tile.TileContext, lo, xb, mb, x0c, m0c, menc):
    nc = tc.nc
    # row = p*(ST*NB) + s*NB + n ; per (s, p): NB*C contiguous bf16 = 8KB lines
    Xd = xb.rearrange("(p s n) c -> s p (n c)", s=ST, n=NB)
    Md = mb.rearrange("(p s n) c -> s p (n c)", s=ST, n=NB)
    LOd = lo.rearrange("(p s n) -> p s n", s=ST, n=NB)
    X0d = x0c.rearrange("(p w) -> p w", w=NSTATS)
    Ed = menc.rearrange("(p s n) c -> s p (n c)", s=ST, n=NB)
    M0d = m0c.rearrange("(p w) -> p w", w=NSTATS)

    import contextlib

    with contextlib.ExitStack() as ctx:
        stats = ctx.enter_context(tc.tile_pool(name="stats", bufs=1))
        work = ctx.enter_context(tc.tile_pool(name="work", bufs=3))
        fpool = ctx.enter_context(tc.tile_pool(name="fpool", bufs=2))
        small = ctx.enter_context(tc.tile_pool(name="small", bufs=2))
        xpool = ctx.enter_context(tc.tile_pool(name="xpool", bufs=3))

        # Persistent per-core stat arrays: S_ALL[q] for q in [M, ME, MX, E]
        S_ALL = stats.tile([P, 4, ST, NB, 2], BF16)  # fold stops at width 2
        X0 = stats.tile([P, NSTATS], BF16)
        M0 = stats.tile([P, NSTATS], BF16)

        lot = small.tile([P, NSTATS], BF16, tag="lot")
        warm = stats.tile([P, 1], BF16)
        # dependency-free first ACT op: forces the exp/ln table load during
        # the DMA ramp instead of stalling the first real exp
        nc.scalar.activation(out=warm, in_=warm, func=AF.Exp)


        # ---- combine (emitted interleaved with the ST loop) ----
        CH_END = [8, 16, 24, 30, 32]  # trigger after this many STs
        WMAX = 8 * NB

        def flat(t):
            return t.rearrange("p a b -> p (a b)")

        def chunk_tiles(ch):
            lo_c = (CH_END[ch - 1] if ch else 0) * NB
            hi_c = CH_END[ch] * NB
            W = hi_c - lo_c
            sl = slice(lo_c, hi_c)

            def tl(tag):
                t = small.tile([P, WMAX], BF16, tag=tag, name=f"cmb{ch}-{tag}")
                return t[:, :W]

            SQ = S_ALL.rearrange("p q s n two -> p q (s n) two")[:, :, sl]
            SQs = small.tile([P, 4, WMAX], BF16, tag="sqs", name=f"cmb{ch}-sqs")[
                :, :, :W
            ]
            return {
                "sl": sl,
                "SQ": SQ,
                "SQs": SQs,
                "sM": SQs[:, 0],
                "sME": SQs[:, 1],
                "sMX": SQs[:, 2],
                "sE": SQs[:, 3],
                "x0": X0[:, sl],
                "m0": M0[:, sl],
                "E0": tl("c0"),
                "t_a": tl("c1"),
                "s_pos": tl("c2"),
                "lse_all": tl("c3"),
                "lse_pos": tl("c4"),
                "lse_neg": tl("c5"),
                "rinv": tl("c7"),
                "s_neg": tl("c8"),
                "npc": tl("c9"),
                "t_b": tl("c11"),
            }

        def combine_pre(t):
            # one ST before the trigger: E0 only (needs just the prefetch)
            nc.scalar.activation(out=t["E0"], in_=t["x0"], func=AF.Exp)

        def combine_v1a1(t):
            # ---- V1: finish the fold (width 2 -> 1) for all four stats
            nc.vector.tensor_add(t["SQs"], t["SQ"][:, :, :, 0], t["SQ"][:, :, :, 1])
            # mask col0 is zeroed on host, so sums exclude class 0
            nc.vector.tensor_add(t["s_pos"], t["sME"], t["E0"])
            nc.vector.tensor_sub(t["s_neg"], t["sE"], t["sME"])
            nc.vector.tensor_tensor(
                out=t["s_neg"], in0=t["s_neg"], in1=t["E0"], op=ALU.max
            )
            nc.vector.tensor_scalar_max(t["npc"], t["sM"], 1.0)
            # ---- A1: the logs, batched
            nc.scalar.activation(out=t["lse_all"], in_=t["sE"], func=AF.Ln)
            nc.scalar.activation(out=t["lse_pos"], in_=t["s_pos"], func=AF.Ln)
            nc.scalar.activation(out=t["lse_neg"], in_=t["s_neg"], func=AF.Ln)
            nc.scalar.activation(out=t["rinv"], in_=t["npc"], func=AF.Ln)
            nc.scalar.activation(out=t["rinv"], in_=t["rinv"], func=AF.Exp, scale=-1.0)

        def combine_v2(ch, t):
            # ---- V2: finish (ACT logs are ~one ST old by now -> no stall)
            ta, tb = t["t_a"], t["t_b"]
            nc.vector.tensor_mul(ta, t["sM"], t["lse_pos"])
            nc.vector.tensor_sub(ta, ta, t["sMX"])
            nc.vector.tensor_mul(ta, ta, t["rinv"])
            nc.vector.tensor_add(ta, ta, t["lse_neg"])  # acc (loss_full sans -x0)
            # lo = acc + m0*(lse_all - acc) - x0   (m0 in {0,1})
            nc.vector.tensor_sub(tb, t["lse_all"], ta)
            nc.vector.tensor_mul(tb, t["m0"], tb)
            nc.vector.tensor_add(ta, ta, tb)
            nc.vector.tensor_sub(lot[:, t["sl"]], ta, t["x0"])
            # stream this chunk's losses out now (contiguous per partition)
            s0 = CH_END[ch - 1] if ch else 0
            nc.sync.dma_start(
                out=LOd[:, s0 : CH_END[ch]],
                in_=lot.rearrange("p (s n) -> p s n", n=NB)[:, s0 : CH_END[ch]],
            )

        def emit_compute(s, W5f, Xp, n0, nn):
            """Compute on row-blocks [n0, n0+nn) of super-tile s."""
            W5 = W5f[:, :, n0 : n0 + nn]
            et = W5[:, 3]
            xbf = W5[:, 4]

            # ScalarE: exp (only needs the xb DMA)
            nc.scalar.activation(out=et, in_=xbf, func=AF.Exp)

            if Xp is None:
                # classic path: both products in one bf16 2x TT
                nc.vector.tensor_mul(
                    W5[:, 1:3], W5[:, 3:5], W5[:, 0:1].broadcast_to([P, 2, nn, C])
                )
            else:
                # hybrid: first half classic 2-wide product; second half gets
                # M*E as exp(x+m_enc) with x' built by a SWDGE compute-add
                # (keeps the extra DMA under the per-ST DVE period)
                h = nn // 2
                nc.vector.tensor_mul(
                    W5[:, 1:3, n0 : n0 + h],
                    W5[:, 3:5, n0 : n0 + h],
                    W5[:, 0:1, n0 : n0 + h].broadcast_to([P, 2, h, C]),
                )
                nc.vector.tensor_mul(
                    W5[:, 2, n0 + h : n0 + nn],
                    W5[:, 0, n0 + h : n0 + nn],
                    W5[:, 4, n0 + h : n0 + nn],
                )
                nc.scalar.activation(
                    out=W5[:, 1, n0 + h : n0 + nn], in_=Xp, func=AF.Exp
                )

            # single fold chain over all four quantities (bf16 2x adds)
            Q = W5[:, 0:4]
            f1 = fpool.tile([P, 4, NB, C // 2], BF16, tag="f1", name="f1")[:, :, n0 : n0 + nn]
            f2 = fpool.tile([P, 4, NB, C // 4], BF16, tag="f2", name="f2")[:, :, n0 : n0 + nn]
            f3 = fpool.tile([P, 4, NB, C // 8], BF16, tag="f3", name="f3")[:, :, n0 : n0 + nn]
            f4 = fpool.tile([P, 4, NB, C // 16], BF16, tag="f4", name="f4")[:, :, n0 : n0 + nn]
            f5 = fpool.tile([P, 4, NB, C // 32], BF16, tag="f5", name="f5")[:, :, n0 : n0 + nn]
            nc.vector.tensor_add(f1, Q[:, :, :, 0 : C // 2], Q[:, :, :, C // 2 : C])
            nc.vector.tensor_add(
                f2, f1[:, :, :, 0 : C // 4], f1[:, :, :, C // 4 : C // 2]
            )
            nc.vector.tensor_add(
                f3, f2[:, :, :, 0 : C // 8], f2[:, :, :, C // 8 : C // 4]
            )
            nc.vector.tensor_add(
                f4, f3[:, :, :, 0 : C // 16], f3[:, :, :, C // 16 : C // 8]
            )
            nc.vector.tensor_add(
                f5, f4[:, :, :, 0 : C // 32], f4[:, :, :, C // 32 : C // 16]
            )
            nc.vector.tensor_add(
                S_ALL[:, :, s, n0 : n0 + nn],
                f5[:, :, :, 0 : C // 64],
                f5[:, :, :, C // 64 : C // 32],
            )

        pending = None  # (ch, tiles) awaiting V2 emission
        tiles_by_ch = {}
        for s in range(ST):
            W5f = work.tile([P, 5, NB, C], BF16, tag="W5", name="W5")
            Xs = Xd[s].rearrange("p (n c) -> p n c", c=C)
            Ms = Md[s].rearrange("p (n c) -> p n c", c=C)
            Es = Ed[s].rearrange("p (n c) -> p n c", c=C)
            if s == 0:
                # progressive first super-tile: tiny DMAs so ACT/DVE start
                # as soon as the first 128KB lands; classic product path
                n0 = 0
                for nn in (2, 2, 4, 8):
                    nc.sync.dma_start(
                        out=W5f[:, 4, n0 : n0 + nn], in_=Xs[:, n0 : n0 + nn]
                    )
                    nc.gpsimd.dma_start(
                        out=W5f[:, 0, n0 : n0 + nn], in_=Ms[:, n0 : n0 + nn]
                    )
                    emit_compute(0, W5f, None, n0, nn)
                    n0 += nn
                # class-0 column prefetch (1KB/partition, contiguous), needed
                # first by chunk 0's E0 at s = CH_END[0]-2
                nc.sync.dma_start(out=X0, in_=X0d)
                nc.sync.dma_start(out=M0, in_=M0d)
            elif s <= 2:
                # keep the classic path while the pipeline warms; the SWDGE
                # x' chains engage from s=3 with full lookahead
                nc.sync.dma_start(out=W5f[:, 4], in_=Xs)
                nc.gpsimd.dma_start(out=W5f[:, 0], in_=Ms)
                emit_compute(s, W5f, None, 0, NB)
            else:
                H = NB // 2
                Xp = xpool.tile([P, H, C], BF16, tag="xp", name="xp")
                nc.sync.dma_start(out=W5f[:, 4], in_=Xs)
                nc.gpsimd.dma_start(out=W5f[:, 0], in_=Ms)
                nc.sync.dma_start(out=Xp, in_=Xs[:, H:NB])
                nc.gpsimd.dma_start(out=Xp, in_=Es[:, H:NB], accum_op=ALU.add)
                emit_compute(s, W5f, Xp, 0, NB)
            if pending is not None:
                combine_v2(*pending)
                pending = None
            if (s + 2) in CH_END:
                ch = CH_END.index(s + 2)
                tiles_by_ch[ch] = chunk_tiles(ch)
                combine_pre(tiles_by_ch[ch])
            if (s + 1) in CH_END:
                ch = CH_END.index(s + 1)
                if ch not in tiles_by_ch:  # back-to-back chunks
                    tiles_by_ch[ch] = chunk_tiles(ch)
                    combine_pre(tiles_by_ch[ch])
                t = tiles_by_ch.pop(ch)
                combine_v1a1(t)
                if s + 1 == ST:
                    combine_v2(ch, t)
                else:
                    pending = (ch, t)
        assert pending is None


def _split_sync_waits(nc):
    """The container's walrus accepts at most ONE sync-wait command per
    instruction (the TPB EVENTS struct has a single wait slot). Tile emits
    instructions with N waits; rewrite each so the extra waits ride on
    same-engine NoOps inserted immediately before (engine program order makes
    this semantically identical)."""
    for f in nc.m.functions:
        for blk in f.blocks:
            insts = blk.instructions
            out = []
            changed = False
            for inst in insts:
                si = inst.sync_info
                waits = list(si.on_wait) if (si is not None and si.on_wait) else []
                if len(waits) > 1:
                    changed = True
                    for k, w in enumerate(waits[:-1]):
                        nop = mybir.InstNoOp(name=f"{inst.name}-w{k}", ins=[], outs=[])
                        nop.engine = inst.engine
                        nop.sync_info = mybir.SyncInfo(on_wait=[w], on_update=[])
                        out.append(nop)
                    inst.sync_info = mybir.SyncInfo(
                        on_wait=[waits[-1]],
                        on_update=list(si.on_update) if si.on_update else [],
                    )
                out.append(inst)
            if changed:
                blk.instructions = out


_NC_CACHE = None
SPLIT_WAITS = True


def _get_nc():
    global _NC_CACHE
    if _NC_CACHE is None:
        nc = bass.Bass()
        xb = nc.declare_dram_parameter("xb", [RPC, C], BF16, isOutput=False)
        mb = nc.declare_dram_parameter("mb", [RPC, C], I8, isOutput=False)
        x0c = nc.declare_dram_parameter("x0c", [RPC], BF16, isOutput=False)
        m0c = nc.declare_dram_parameter("m0c", [RPC], BF16, isOutput=False)
        menc = nc.declare_dram_parameter("menc", [RPC, C], I8, isOutput=False)
        lo = nc.declare_dram_parameter("lo", [RPC], BF16, isOutput=True)
        with tile.TileContext(nc) as tc:
            _build_kernel(tc, lo, xb, mb, x0c, m0c, menc)
        if SPLIT_WAITS:
            _split_sync_waits(nc)
        _NC_CACHE = nc
    return _NC_CACHE


def _prep_inputs(logit, target):
    """Host-side re-encoding (dtype casts + column slice/zero only)."""
    xb = logit.astype(ml_dtypes.bfloat16)
    mb = target.astype(np.int8)
    x0c = np.ascontiguousarray(xb[:, 0])
    m0c = target[:, 0].astype(ml_dtypes.bfloat16)
    mb[:, 0] = 0
    menc = ((target - 1) * 32).astype(np.int8)
    menc[:, 0] = -32
    return xb, mb, x0c, m0c, menc


def _in_maps(xb, mb, x0c, m0c, menc):
    return [
        {
            "xb": xb[i * RPC : (i + 1) * RPC],
            "mb": mb[i * RPC : (i + 1) * RPC],
            "x0c": x0c[i * RPC : (i + 1) * RPC],
            "m0c": m0c[i * RPC : (i + 1) * RPC],
            "menc": menc[i * RPC : (i + 1) * RPC],
        }
        for i in range(NCORES)
    ]


def kernel(**inputs) -> np.ndarray:
    logit = np.ascontiguousarray(np.asarray(inputs["logit"], dtype=np.float32))
    target = np.ascontiguousarray(np.asarray(inputs["target"], dtype=np.int32))
    assert logit.shape == (B, C) and target.shape == (B, C)

    nc = _get_nc()
    res = run_bass_kernel_spmd(
        nc, _in_maps(*_prep_inputs(logit, target)), core_ids=list(range(NCORES))
    )
    lo = np.concatenate(
        [np.asarray(r["lo"]).reshape(-1) for r in res.results]
    ).astype(np.float32)
    return np.array(np.mean(lo, dtype=np.float64), dtype=np.float32)


# revision 24
# speedup vs baseline: 1.0314x; 1.0091x over previous
"""Trainium2 Bass kernel for the masked-logsumexp multi-branch loss (final).

Problem: logit [524288, 128] f32, target [524288, 128] int32 (multi-hot 0/1).
Per row r (is_null = target[r,0]):
  branch1 (is_null): lse(all) - x0
  branch2: (n_pos*lse({0} u pos) - sum_pos_logit)/max(n_pos,1) + lse(neg u {0}) - x0
Output: scalar mean over all rows. Data-parallel over 8 NeuronCores (65536
rows/core), rows on SBUF partitions; per-row sums shared across the three
masked logsumexps: S_E, S_ME, S_MX, S_M plus class-0 extracts.

Host re-encoding (casts/slices/affine of single tensors only): logit/target
-> bf16; class-0 columns as separate [B] tensors; mask column 0 zeroed
(ships as (mask-sans-col0, col0), mirroring the reference's pos_no0); and
m_enc = (target-1)*32 in int8 ({0,-32}, col0=-32). Row mapping is
partition-major (row = p*512 + s*NB + n) so column tensors and the lo
output are contiguous per partition.

Per [128, 16*128] super-tile (NB=16, 32/core):
  DMA(HWDGE): xb -> W5[:,4]; 2nd read of xb's upper half into Xp.
  DMA(SWDGE): mask int8 -> bf16 casting load into W5[:,0] (saves 8MB/core
              of HBM); Xp += m_enc (compute-copy cast+add from int8,
              4KB/partition descriptors) -- an independent chain joining
              only at the fold.
  ScalarE  : exp(xb) -> W5[:,3]; exp(Xp) -> W5[:,1] upper half
             (exp(x+m_enc) == M*exp(x) exactly on kept terms, ~1e-14 leak
             on masked terms).
  VectorE  : lower half: one 2-wide bf16 2x TT for [M*E|M*x]; upper half:
             M*x only (the M*E product is what the SWDGE chain replaced);
             then a shared 2x fold chain over [mb|M*E|M*x|E] to width 2,
             landing in the bf16 stat array. The CCE covers only half the
             stream because its RMW runs ~4x below line rate: full-stream
             CCE starves the pipeline (measured 263-287us), zero-CCE leaves
             DVE product work on the floor (250us); half splits the load so
             per-ST DMA time stays under per-ST DVE time.
  (GPSIMD compute and TensorE unusable: Pool shares the DVE SBUF port and
   is ~2x slower per element; PE only contracts the partition axis and PSUM
   accumulation is matmul-only; tensor_reduce/pool/TTR/custom-DVE run at 1x
   < the 2x fold chain.)
Super-tile 0 is emitted in progressive pieces (2/2/4/8 blocks) and STs 1-2
stay on the classic product path so the SWDGE chains engage with full
lookahead. Combine runs per chunk in bf16, split E0-early / V1+A1 at the
trigger / V2 one ST later so the DVE never waits on the ACT logs; each
chunk streams its bf16 losses out immediately. Final mean on host (f64).

Measured on trn2 (8 cores): ~238-242us HW exec (best 237.9us) (vs 262.5us baseline; Vector
~211us busy, ~10.6us engine preamble, ~3us gaps, ~5us tail), output rel
err 3.3e-4 vs the 2e-2 tolerance (validated bit-exact in numpy first).
Also rejected on measurement: NB=32 tiles (DVE throughput -4% at 8KB FD),
SWDGE int8 mask-fold (descriptor-bound at 8-64B chunks: 633us), CCE on
5/8 of the stream with or without the int8 mask (241-246us: the SWDGE
cost grows faster than the DVE saving past the half split), and a 4th
Xp buffer (244us).

Note: this container's walrus accepts one sync-wait per instruction, so
_split_sync_waits() rewrites the Tile-scheduled BIR accordingly.
"""
import numpy as np
import ml_dtypes

import concourse.bass as bass
import concourse.tile as tile
from concourse import mybir
from concourse.bass_utils import run_bass_kernel_spmd

B = 524288
C = 128
NCORES = 8
RPC = B // NCORES  # rows per core = 65536
P = 128  # partitions
NB = 16  # class-blocks per super-tile -> [128, NB*128] tiles
ST = RPC // (P * NB)  # super-tiles per core = 16
NSTATS = ST * NB  # stat columns per core = 512

FP32 = mybir.dt.float32
BF16 = mybir.dt.bfloat16
I8 = mybir.dt.int8
ALU = mybir.AluOpType
AF = mybir.ActivationFunctionType


def _build_kernel(tc: tile.TileContext, lo, xb, mb, x0c, m0c, menc):
    nc = tc.nc
    # row = p*(ST*NB) + s*NB + n ; per (s, p): NB*C contiguous bf16 = 8KB lines
    Xd = xb.rearrange("(p s n) c -> s p (n c)", s=ST, n=NB)
    Md = mb.rearrange("(p s n) c -> s p (n c)", s=ST, n=NB)
    LOd = lo.rearrange("(p s n) -> p s n", s=ST, n=NB)
    X0d = x0c.rearrange("(p w) -> p w", w=NSTATS)
    Ed = menc.rearrange("(p s n) c -> s p (n c)", s=ST, n=NB)
    M0d = m0c.rearrange("(p w) -> p w", w=NSTATS)

    import contextlib

    with contextlib.ExitStack() as ctx:
        stats = ctx.enter_context(tc.tile_pool(name="stats", bufs=1))
        work = ctx.enter_context(tc.tile_pool(name="work", bufs=3))
        fpool = ctx.enter_context(tc.tile_pool(name="fpool", bufs=2))
        small = ctx.enter_context(tc.tile_pool(name="small", bufs=2))
        xpool = ctx.enter_context(tc.tile_pool(name="xpool", bufs=3))

        # Persistent per-core stat arrays: S_ALL[q] for q in [M, ME, MX, E]
        S_ALL = stats.tile([P, 4, ST, NB, 2], BF16)  # fold stops at width 2
        X0 = stats.tile([P, NSTATS], BF16)
        M0 = stats.tile([P, NSTATS], BF16)

        lot = small.tile([P, NSTATS], BF16, tag="lot")
        warm = stats.tile([P, 1], BF16)
        # dependency-free first ACT op: forces the exp/ln table load during
        # the DMA ramp instead of stalling the first real exp
        nc.scalar.activation(out=warm, in_=warm, func=AF.Exp)


        # ---- combine (emitted interleaved with the ST loop) ----
        CH_END = [8, 16, 24, 30, 32]  # trigger after this many STs
        WMAX = 8 * NB

        def flat(t):
            return t.rearrange("p a b -> p (a b)")

        def chunk_tiles(ch):
            lo_c = (CH_END[ch - 1] if ch else 0) * NB
            hi_c = CH_END[ch] * NB
            W = hi_c - lo_c
            sl = slice(lo_c, hi_c)

            def tl(tag):
                t = small.tile([P, WMAX], BF16, tag=tag, name=f"cmb{ch}-{tag}")
                return t[:, :W]

            SQ = S_ALL.rearrange("p q s n two -> p q (s n) two")[:, :, sl]
            SQs = small.tile([P, 4, WMAX], BF16, tag="sqs", name=f"cmb{ch}-sqs")[
                :, :, :W
            ]
            return {
                "sl": sl,
                "SQ": SQ,
                "SQs": SQs,
                "sM": SQs[:, 0],
                "sME": SQs[:, 1],
                "sMX": SQs[:, 2],
                "sE": SQs[:, 3],
                "x0": X0[:, sl],
                "m0": M0[:, sl],
                "E0": tl("c0"),
                "t_a": tl("c1"),
                "s_pos": tl("c2"),
                "lse_all": tl("c3"),
                "lse_pos": tl("c4"),
                "lse_neg": tl("c5"),
                "rinv": tl("c7"),
                "s_neg": tl("c8"),
                "npc": tl("c9"),
                "t_b": tl("c11"),
            }

        def combine_pre(t):
            # one ST before the trigger: E0 only (needs just the prefetch)
            nc.scalar.activation(out=t["E0"], in_=t["x0"], func=AF.Exp)

        def combine_v1a1(t):
            # ---- V1: finish the fold (width 2 -> 1) for all four stats
            nc.vector.tensor_add(t["SQs"], t["SQ"][:, :, :, 0], t["SQ"][:, :, :, 1])
            # mask col0 is zeroed on host, so sums exclude class 0
            nc.vector.tensor_add(t["s_pos"], t["sME"], t["E0"])
            nc.vector.tensor_sub(t["s_neg"], t["sE"], t["sME"])
            nc.vector.tensor_tensor(
                out=t["s_neg"], in0=t["s_neg"], in1=t["E0"], op=ALU.max
            )
            nc.vector.tensor_scalar_max(t["npc"], t["sM"], 1.0)
            # ---- A1: the logs, batched
            nc.scalar.activation(out=t["lse_all"], in_=t["sE"], func=AF.Ln)
            nc.scalar.activation(out=t["lse_pos"], in_=t["s_pos"], func=AF.Ln)
            nc.scalar.activation(out=t["lse_neg"], in_=t["s_neg"], func=AF.Ln)
            nc.scalar.activation(out=t["rinv"], in_=t["npc"], func=AF.Ln)
            nc.scalar.activation(out=t["rinv"], in_=t["rinv"], func=AF.Exp, scale=-1.0)

        def combine_v2(ch, t):
            # ---- V2: finish (ACT logs are ~one ST old by now -> no stall)
            ta, tb = t["t_a"], t["t_b"]
            nc.vector.tensor_mul(ta, t["sM"], t["lse_pos"])
            nc.vector.tensor_sub(ta, ta, t["sMX"])
            nc.vector.tensor_mul(ta, ta, t["rinv"])
            nc.vector.tensor_add(ta, ta, t["lse_neg"])  # acc (loss_full sans -x0)
            # lo = acc + m0*(lse_all - acc) - x0   (m0 in {0,1})
            nc.vector.tensor_sub(tb, t["lse_all"], ta)
            nc.vector.tensor_mul(tb, t["m0"], tb)
            nc.vector.tensor_add(ta, ta, tb)
            nc.vector.tensor_sub(lot[:, t["sl"]], ta, t["x0"])
            # stream this chunk's losses out now (contiguous per partition)
            s0 = CH_END[ch - 1] if ch else 0
            nc.sync.dma_start(
                out=LOd[:, s0 : CH_END[ch]],
                in_=lot.rearrange("p (s n) -> p s n", n=NB)[:, s0 : CH_END[ch]],
            )

        def emit_compute(s, W5f, Xp, n0, nn):
            """Compute on row-blocks [n0, n0+nn) of super-tile s."""
            W5 = W5f[:, :, n0 : n0 + nn]
            et = W5[:, 3]
            xbf = W5[:, 4]

            # ScalarE: exp (only needs the xb DMA)
            nc.scalar.activation(out=et, in_=xbf, func=AF.Exp)

            if Xp is None:
                # classic path: both products in one bf16 2x TT
                nc.vector.tensor_mul(
                    W5[:, 1:3], W5[:, 3:5], W5[:, 0:1].broadcast_to([P, 2, nn, C])
                )
            else:
                # hybrid: first half classic 2-wide product; second half gets
                # M*E as exp(x+m_enc) with x' built by a SWDGE compute-add
                # (keeps the extra DMA under the per-ST DVE period)
                h = nn // 2
                nc.vector.tensor_mul(
                    W5[:, 1:3, n0 : n0 + h],
                    W5[:, 3:5, n0 : n0 + h],
                    W5[:, 0:1, n0 : n0 + h].broadcast_to([P, 2, h, C]),
                )
                nc.vector.tensor_mul(
                    W5[:, 2, n0 + h : n0 + nn],
                    W5[:, 0, n0 + h : n0 + nn],
                    W5[:, 4, n0 + h : n0 + nn],
                )
                nc.scalar.activation(
                    out=W5[:, 1, n0 + h : n0 + nn], in_=Xp, func=AF.Exp
                )

            # single fold chain over all four quantities (bf16 2x adds)
            Q = W5[:, 0:4]
            f1 = fpool.tile([P, 4, NB, C // 2], BF16, tag="f1", name="f1")[:, :, n0 : n0 + nn]
            f2 = fpool.tile([P, 4, NB, C // 4], BF16, tag="f2", name="f2")[:, :, n0 : n0 + nn]
            f3 = fpool.tile([P, 4, NB, C // 8], BF16, tag="f3", name="f3")[:, :, n0 : n0 + nn]
            f4 = fpool.tile([P, 4, NB, C // 16], BF16, tag="f4", name="f4")[:, :, n0 : n0 + nn]
            f5 = fpool.tile([P, 4, NB, C // 32], BF16, tag="f5", name="f5")[:, :, n0 : n0 + nn]
            nc.vector.tensor_add(f1, Q[:, :, :, 0 : C // 2], Q[:, :, :, C // 2 : C])
            nc.vector.tensor_add(
                f2, f1[:, :, :, 0 : C // 4], f1[:, :, :, C // 4 : C // 2]
            )
            nc.vector.tensor_add(
                f3, f2[:, :, :, 0 : C // 8], f2[:, :, :, C // 8 : C // 4]
            )
            nc.vector.tensor_add(
                f4, f3[:, :, :, 0 : C // 16], f3[:, :, :, C // 16 : C // 8]
            )
            nc.vector.tensor_add(
                f5, f4[:, :, :, 0 : C // 32], f4[:, :, :, C // 32 : C // 16]
            )
            nc.vector.tensor_add(
                S_ALL[:, :, s, n0 : n0 + nn],
                f5[:, :, :, 0 : C // 64],
                f5[:, :, :, C // 64 : C // 32],
            )

        pending = None  # (ch, tiles) awaiting V2 emission
        tiles_by_ch = {}
        for s in range(ST):
            W5f = work.tile([P, 5, NB, C], BF16, tag="W5", name="W5")
            Xs = Xd[s].rearrange("p (n c) -> p n c", c=C)
            Ms = Md[s].rearrange("p (n c) -> p n c", c=C)
            Es = Ed[s].rearrange("p (n c) -> p n c", c=C)
            if s == 0:
                # progressive first super-tile: tiny DMAs so ACT/DVE start
                # as soon as the first 128KB lands; classic product path
                n0 = 0
                for nn in (1, 1, 2, 4, 8):
                    nc.sync.dma_start(
                        out=W5f[:, 4, n0 : n0 + nn], in_=Xs[:, n0 : n0 + nn]
                    )
                    nc.gpsimd.dma_start(
                        out=W5f[:, 0, n0 : n0 + nn], in_=Ms[:, n0 : n0 + nn]
                    )
                    emit_compute(0, W5f, None, n0, nn)
                    n0 += nn
                # class-0 column prefetch (1KB/partition, contiguous), needed
                # first by chunk 0's E0 at s = CH_END[0]-2
                nc.sync.dma_start(out=X0, in_=X0d)
                nc.sync.dma_start(out=M0, in_=M0d)
            elif s <= 2:
                # keep the classic path while the pipeline warms; the SWDGE
                # x' chains engage from s=3 with full lookahead
                nc.sync.dma_start(out=W5f[:, 4], in_=Xs)
                nc.gpsimd.dma_start(out=W5f[:, 0], in_=Ms)
                emit_compute(s, W5f, None, 0, NB)
            else:
                H = NB // 2
                Xp = xpool.tile([P, H, C], BF16, tag="xp", name="xp")
                nc.sync.dma_start(out=W5f[:, 4], in_=Xs)
                nc.gpsimd.dma_start(out=W5f[:, 0], in_=Ms)
                nc.sync.dma_start(out=Xp, in_=Xs[:, H:NB])
                nc.gpsimd.dma_start(out=Xp, in_=Es[:, H:NB], accum_op=ALU.add)
                emit_compute(s, W5f, Xp, 0, NB)
            if pending is not None:
                combine_v2(*pending)
                pending = None
            if (s + 2) in CH_END:
                ch = CH_END.index(s + 2)
                tiles_by_ch[ch] = chunk_tiles(ch)
                combine_pre(tiles_by_ch[ch])
            if (s + 1) in CH_END:
                ch = CH_END.index(s + 1)
                if ch not in tiles_by_ch:  # back-to-back chunks
                    tiles_by_ch[ch] = chunk_tiles(ch)
                    combine_pre(tiles_by_ch[ch])
                t = tiles_by_ch.pop(ch)
                combine_v1a1(t)
                if s + 1 == ST:
                    combine_v2(ch, t)
                else:
                    pending = (ch, t)
        assert pending is None


def _split_sync_waits(nc):
    """The container's walrus accepts at most ONE sync-wait command per
    instruction (the TPB EVENTS struct has a single wait slot). Tile emits
    instructions with N waits; rewrite each so the extra waits ride on
    same-engine NoOps inserted immediately before (engine program order makes
    this semantically identical)."""
    for f in nc.m.functions:
        for blk in f.blocks:
            insts = blk.instructions
            out = []
            changed = False
            for inst in insts:
                si = inst.sync_info
                waits = list(si.on_wait) if (si is not None and si.on_wait) else []
                if len(waits) > 1:
                    changed = True
                    for k, w in enumerate(waits[:-1]):
                        nop = mybir.InstNoOp(name=f"{inst.name}-w{k}", ins=[], outs=[])
                        nop.engine = inst.engine
                        nop.sync_info = mybir.SyncInfo(on_wait=[w], on_update=[])
                        out.append(nop)
                    inst.sync_info = mybir.SyncInfo(
                        on_wait=[waits[-1]],
                        on_update=list(si.on_update) if si.on_update else [],
                    )
                out.append(inst)
            if changed:
                blk.instructions = out


_NC_CACHE = None
SPLIT_WAITS = True


def _get_nc():
    global _NC_CACHE
    if _NC_CACHE is None:
        nc = bass.Bass()
        xb = nc.declare_dram_parameter("xb", [RPC, C], BF16, isOutput=False)
        mb = nc.declare_dram_parameter("mb", [RPC, C], I8, isOutput=False)
        x0c = nc.declare_dram_parameter("x0c", [RPC], BF16, isOutput=False)
        m0c = nc.declare_dram_parameter("m0c", [RPC], BF16, isOutput=False)
        menc = nc.declare_dram_parameter("menc", [RPC, C], I8, isOutput=False)
        lo = nc.declare_dram_parameter("lo", [RPC], BF16, isOutput=True)
        with tile.TileContext(nc) as tc:
            _build_kernel(tc, lo, xb, mb, x0c, m0c, menc)
        if SPLIT_WAITS:
            _split_sync_waits(nc)
        _NC_CACHE = nc
    return _NC_CACHE


def _prep_inputs(logit, target):
    """Host-side re-encoding (dtype casts + column slice/zero only)."""
    xb = logit.astype(ml_dtypes.bfloat16)
    mb = target.astype(np.int8)
    x0c = np.ascontiguousarray(xb[:, 0])
    m0c = target[:, 0].astype(ml_dtypes.bfloat16)
    mb[:, 0] = 0
    menc = ((target - 1) * 32).astype(np.int8)
    menc[:, 0] = -32
    return xb, mb, x0c, m0c, menc


def _in_maps(xb, mb, x0c, m0c, menc):
    return [
        {
            "xb": xb[i * RPC : (i + 1) * RPC],
            "mb": mb[i * RPC : (i + 1) * RPC],
            "x0c": x0c[i * RPC : (i + 1) * RPC],
            "m0c": m0c[i * RPC : (i + 1) * RPC],
            "menc": menc[i * RPC : (i + 1) * RPC],
        }
        for i in range(NCORES)
    ]


def kernel(**inputs) -> np.ndarray:
    logit = np.ascontiguousarray(np.asarray(inputs["logit"], dtype=np.float32))
    target = np.ascontiguousarray(np.asarray(inputs["target"], dtype=np.int32))
    assert logit.shape == (B, C) and target.shape == (B, C)

    nc = _get_nc()
    res = run_bass_kernel_spmd(
        nc, _in_maps(*_prep_inputs(logit, target)), core_ids=list(range(NCORES))
    )
    lo = np.concatenate(
        [np.asarray(r["lo"]).reshape(-1) for r in res.results]
    ).astype(np.float32)
    return np.array(np.mean(lo, dtype=np.float64), dtype=np.float32)
